# revision 23
# baseline (speedup 1.0000x reference)
import sys

sys.path.insert(0, "/opt/trn_rl_repo")

from contextlib import ExitStack

import numpy as np
import ml_dtypes
import concourse.bacc as bacc
import concourse.mybir as mybir
from concourse.bass_utils import run_bass_kernel_spmd
from concourse.tile import TileContext
from concourse.masks import make_identity

P = 128
NCORES = 8
N, D, E, KHOP, B, L = 100000, 128, 1600000, 3, 32768, 262144
H_MLP, R = 512, 64
SHARD = 12544           # nodes per core (98 * 128); core 7 padded
NP = NCORES * SHARD     # 100352
NPAIR = SHARD // 2      # 6272 pair-rows per core
NSUP = NP // 256        # 392 dst superblocks (256 nodes each)
BSEG = B // NCORES      # 4096 segments per core
BSUP = B // 256         # 128 seg superblocks
NTILE = SHARD // 256    # 49 pair tiles per core
NSUPO = NSUP // NCORES  # 49 dst superblocks per owner
SLICES = [8, 8, 8, 8, 8, 8, 1]   # hop RS slices, per-owner supers (sum 49)
NSPLIT = len(SLICES)
SCUM = [0]
for _s in SLICES:
    SCUM.append(SCUM[-1] + _s)       # per-owner cumulative
SUPCUM = [c * NCORES for c in SCUM]  # global super boundaries
PSLICES = [5, 5, 5, 1]               # pool RS slices, per-owner supers (sum 16)
NSPLITP = len(PSLICES)
PCUM = [0]
for _s in PSLICES:
    PCUM.append(PCUM[-1] + _s)
PSUPCUM = [c * NCORES for c in PCUM]
SUPB = {SUPCUM[j + 1]: j for j in range(NSPLIT)}       # slice end -> idx
SUPTRIG = {min(SUPCUM[j] + 28, SUPCUM[j + 1]) - 1: j   # deferred-coll point
           for j in range(1, NSPLIT)}
PSUPB = {PSUPCUM[j + 1]: j for j in range(NSPLITP)}
PSUPTRIG = {min(PSUPCUM[j] + 20, PSUPCUM[j + 1]) - 1: j
            for j in range(1, NSPLITP)}
QHOP = 8                # gsz quantum for hop schedule
QPOOL = 16              # gsz quantum for pool schedule
GB = 3072               # gather batch tokens (hops)
GBP = 1024              # gather batch tokens (pools)
EG = 4                  # supers per evac DMA group
JU = 256                # mm index at which prior hop's update is emitted

f32 = mybir.dt.float32
bf16 = mybir.dt.bfloat16
i16 = mybir.dt.int16
i32 = mybir.dt.int32

_COMPILED = {}


def _wrap_idx16(idx):
    """dma_gather index layout: token i -> partition i%16, col i//16, x8 replicated."""
    n = len(idx)
    assert n % 16 == 0
    return np.tile(idx.reshape(n // 16, 16).T.astype(np.int16), (8, 1))


def _schedule(bucket_by_core, slot_by_core, idx_by_core, w_by_core,
              nbuckets, quantum):
    """Static SPMD schedule for one-hot scatter matmuls.

    Tokens are sorted by bucket; each bucket's region is padded to `quantum`
    (shared across cores via max count). Chunks of 128 tokens may straddle
    bucket boundaries; straddling chunks get one matmul per bucket touched,
    with masked loc columns.

    Returns (T, mm, idx_streams, loc_tables, w_tables):
      T: padded token count (mult of 128)
      mm: list of (chunk, bucket, start, stop)
      idx_streams[c]: int16 [T] gather indices (pad -> 0)
      loc_tables[c]: f32 [128, nmm] per-matmul slot columns (pad/mask -> -1)
      w_tables[c]: f32 [128, nmm] per-matmul token weights (or None)
    """
    ncores = len(bucket_by_core)
    counts = np.zeros((ncores, nbuckets), np.int64)
    for c in range(ncores):
        np.add.at(counts[c], bucket_by_core[c], 1)
    gsz = ((counts.max(axis=0) + quantum - 1) // quantum) * quantum
    # every bucket must get at least one matmul so its psum slice is
    # written (zeros) before the evacuation reads it
    gsz[gsz == 0] = quantum
    starts = np.zeros(nbuckets + 1, np.int64)
    starts[1:] = np.cumsum(gsz)
    T = int(((starts[-1] + P - 1) // P) * P)

    # static matmul descriptors
    mm = []
    for b in range(nbuckets):
        c0 = int(starts[b]) // P
        c1 = int(starts[b + 1] - 1) // P
        for ch in range(c0, c1 + 1):
            mm.append([ch, b, ch == c0, ch == c1])
    nmm = len(mm)

    idx_streams, loc_tables, w_tables = [], [], []
    for c in range(ncores):
        order = np.argsort(bucket_by_core[c], kind="stable")
        bs = bucket_by_core[c][order]
        sl = slot_by_core[c][order].astype(np.float32)
        ix = idx_by_core[c][order].astype(np.int16)
        run_start = np.concatenate([[0], np.cumsum(counts[c])])
        pos_in_run = np.arange(len(bs)) - run_start[bs]
        out_pos = starts[bs] + pos_in_run
        idx_full = np.zeros(T, np.int16)
        idx_full[out_pos] = ix
        slot_full = np.full(T, -1.0, np.float32)
        slot_full[out_pos] = sl
        bkt_full = np.full(T, -1, np.int64)
        bkt_full[out_pos] = bs
        w_full = np.zeros(T, np.float32)
        if w_by_core is not None:
            w_full[out_pos] = w_by_core[c][order].astype(np.float32)
        # build per-mm loc columns: tokens of chunk ch masked to bucket b
        loc_cols = np.full((nmm, P), -1.0, np.float32)
        w_cols = np.zeros((nmm, P), np.float32)
        for j, (ch, b, _s, _e) in enumerate(mm):
            tok = slice(ch * P, ch * P + P)
            m = bkt_full[tok] == b
            col = np.full(P, -1.0, np.float32)
            col[m] = slot_full[tok][m]
            loc_cols[j] = col
            w_cols[j] = w_full[tok]
        idx_streams.append(idx_full)
        loc_tables.append(np.ascontiguousarray(loc_cols.T))  # [128, nmm]
        w_tables.append(np.ascontiguousarray(w_cols.T))
    return T, mm, idx_streams, loc_tables, w_tables


def _slice_rank(nsup, npo, cum, supcum):
    """Super processing order (slice-major, owner-interleaved) so RS slice j
    covers supers [supcum[j], supcum[j+1]) = every owner's cum[j]..cum[j+1]
    supers, and RS output rows land in natural per-owner order."""
    s = np.arange(nsup)
    o, w = s // npo, s % npo
    j = np.searchsorted(np.asarray(cum), w, side="right") - 1
    pos = w - np.asarray(cum)[j]
    sz = np.asarray(cum)[j + 1] - np.asarray(cum)[j]
    return np.asarray(supcum)[j] + o * sz + pos


def _rank_of_sup():
    return _slice_rank(NSUP, NSUPO, SCUM, SUPCUM)


def _rank_of_psup():
    return _slice_rank(BSUP, BSUP // NCORES, PCUM, PSUPCUM)


def _make_schedules(edge_index, H_idx, H_seg, T_idx, T_seg):
    src = edge_index[0].astype(np.int64)
    dst = edge_index[1].astype(np.int64)

    deg_dst = np.bincount(dst, minlength=NP)
    deg_src = np.bincount(src, minlength=NP)
    a_sc = 1.0 / np.sqrt(np.maximum(deg_src, 1.0))
    b_sc = 1.0 / np.sqrt(np.maximum(deg_dst, 1.0))
    absc = (a_sc * b_sc).astype(np.float32)
    a_sc = a_sc.astype(np.float32)

    rank = _rank_of_sup()
    hb, hs, hi, hw0, hw12 = [], [], [], [], []
    for c in range(NCORES):
        lo = c * SHARD
        m = (src >= lo) & (src < lo + SHARD)
        d = dst[m]
        s_ = src[m]
        hb.append((rank[d >> 8] * 2 + (d & 1)).astype(np.int64))
        hs.append((d >> 1) & 127)
        hi.append(s_ - lo)
        hw0.append(a_sc[s_])
        hw12.append(absc[s_])
    TH, mmH, hidxs, hlocs, hw0s = _schedule(hb, hs, hi, hw0, NSUP * 2, QHOP)
    _, _, _, _, hw12s = _schedule(hb, hs, hi, hw12, NSUP * 2, QHOP)

    prank = _rank_of_psup()

    def pool_sched(idx, seg):
        pb, psl, pii = [], [], []
        idx = idx.astype(np.int64)
        seg = seg.astype(np.int64)
        for c in range(NCORES):
            lo = c * SHARD
            m = (idx >= lo) & (idx < lo + SHARD)
            sg = seg[m]
            pb.append((prank[sg >> 8] * 2 + ((sg >> 7) & 1)).astype(np.int64))
            psl.append(sg & 127)
            pii.append(idx[m] - lo)
        return _schedule(pb, psl, pii, None, BSUP * 2, QPOOL)

    TPh, mmPh, pidxsH, plocsH, _ = pool_sched(H_idx, H_seg)
    TPt, mmPt, pidxsT, plocsT, _ = pool_sched(T_idx, T_seg)
    return dict(TH=TH, mmH=mmH, hidxs=hidxs, hlocs=hlocs, hw0s=hw0s,
                hw12s=hw12s, TPh=TPh, mmPh=mmPh, pidxsH=pidxsH,
                plocsH=plocsH, TPt=TPt, mmPt=mmPt, pidxsT=pidxsT,
                plocsT=plocsT, b_sc=b_sc.astype(np.float32))


def _build_program(TH, mmH, nmmPh, mmPh, nmmPt, mmPt, TPh, TPt):
    nc = bacc.Bacc("TRN2", target_bir_lowering=False, num_devices=NCORES)

    nmmH = len(mmH)

    # ---------------- dram tensors ----------------
    embed_bf = nc.dram_tensor("embed_bf", [SHARD, D], bf16, kind="ExternalInput")
    bsc_in = nc.dram_tensor("bsc_in", [P, NTILE, 2], f32, kind="ExternalInput")
    temp_in = nc.dram_tensor("temp_in", [P, 4], f32, kind="ExternalInput")
    wrep_in = nc.dram_tensor("wrep_in", [P, 2 * D], f32, kind="ExternalInput")
    w1_in = nc.dram_tensor("w1_in", [3, P, H_MLP], bf16, kind="ExternalInput")
    b1_in = nc.dram_tensor("b1_in", [P, 4], f32, kind="ExternalInput")
    w2_in = nc.dram_tensor("w2_in", [4, P, R], bf16, kind="ExternalInput")
    b2_in = nc.dram_tensor("b2_in", [R, 1], f32, kind="ExternalInput")
    hsrc = nc.dram_tensor("hsrc", [P, TH // 16], i16, kind="ExternalInput")
    hloc = nc.dram_tensor("hloc", [P, nmmH], f32, kind="ExternalInput")
    hw0_in = nc.dram_tensor("hw0", [P, nmmH], f32, kind="ExternalInput")
    hw12_in = nc.dram_tensor("hw12", [P, nmmH], f32, kind="ExternalInput")
    psrcH = nc.dram_tensor("psrcH", [P, TPh // 16], i16, kind="ExternalInput")
    plocH = nc.dram_tensor("plocH", [P, nmmPh], f32, kind="ExternalInput")
    psrcT = nc.dram_tensor("psrcT", [P, TPt // 16], i16, kind="ExternalInput")
    plocT = nc.dram_tensor("plocT", [P, nmmPt], f32, kind="ExternalInput")

    out = nc.dram_tensor("out", [BSEG, R], f32, kind="ExternalOutput")

    rg = [list(range(NCORES))]

    with TileContext(nc) as tc, ExitStack() as ctx:
        sb = ctx.enter_context(tc.tile_pool(name="sb", bufs=2))
        dram = ctx.enter_context(tc.tile_pool(name="dram", bufs=1,
                                              space="DRAM"))
        xparts = [dram.tile([NP // 2, 2 * D], bf16, tag=f"xpart{k}",
                            name=f"xpart{k}") for k in range(KHOP)]
        xnews = [dram.tile([SHARD, D], bf16, tag=f"xnew{k}",
                           name=f"xnew{k}") for k in range(KHOP)]
        zext = dram.tile([SHARD, 2 * D], bf16, tag="zext", name="zext")
        ppart = dram.tile([B, 260], bf16, tag="ppart", name="ppart")
        pout = dram.tile([BSEG, 260], bf16, tag="pout", name="pout")
        const = ctx.enter_context(tc.tile_pool(name="const", bufs=1))
        gpool = ctx.enter_context(tc.tile_pool(name="gath", bufs=4))
        gppool = ctx.enter_context(tc.tile_pool(name="gathp", bufs=2))
        ohp = ctx.enter_context(tc.tile_pool(name="ohp", bufs=2))
        def coll(ins_ap, outs_ap):
            """Async ReduceScatter; DRAM-tile dependency tracking orders it
            after the producing DMAs and before consumers."""
            nc.gpsimd.collective_compute(
                "ReduceScatter", mybir.AluOpType.add,
                ins=[ins_ap], outs=[outs_ap], replica_groups=rg,
            )

        # hop gather index/loc/weight tables first: hop-0 work depends on them
        hop_idx = const.tile([P, TH // 16], i16)
        nc.sync.dma_start(hop_idx[:], hsrc[:])
        hop_loc = const.tile([P, nmmH], f32)
        nc.sync.dma_start(hop_loc[:], hloc[:])
        hop_w0 = const.tile([P, nmmH], f32)
        nc.sync.dma_start(hop_w0[:], hw0_in[:])
        hop_w12 = const.tile([P, nmmH], f32)
        nc.sync.dma_start(hop_w12[:], hw12_in[:])

        # ---------------- constants ----------------
        iota_i = const.tile([P, P], i32)
        nc.gpsimd.iota(iota_i[:], pattern=[[1, P]], base=0, channel_multiplier=0)
        iota_b = const.tile([P, P], bf16)
        nc.vector.tensor_copy(iota_b[:], iota_i[:])
        ident = const.tile([P, P], f32)
        make_identity(nc, ident[:])

        temp_sb = const.tile([P, 4], f32)
        nc.sync.dma_start(temp_sb[:], temp_in[:])
        bsc_sb = const.tile([P, NTILE, 2], f32)
        nc.sync.dma_start(bsc_sb[:], bsc_in[:])
        wrep = const.tile([P, 2 * D], f32)
        nc.sync.dma_start(wrep[:], wrep_in[:])
        w1t = const.tile([P, 3, H_MLP], bf16)
        nc.sync.dma_start(w1t[:], w1_in.rearrange("k p h -> p k h")[:])
        b1t = const.tile([P, 4], f32)
        nc.sync.dma_start(b1t[:], b1_in[:])
        w2t = const.tile([P, 4, R], bf16)
        nc.sync.dma_start(w2t[:], w2_in.rearrange("k p r -> p k r")[:])
        b2t = const.tile([R, 1], f32)
        nc.sync.dma_start(b2t[:], b2_in[:])

        # pool tables (needed much later; loaded early, DMA is cheap)
        pool_idx_h = const.tile([P, TPh // 16], i16)
        nc.sync.dma_start(pool_idx_h[:], psrcH[:])
        pool_loc_h = const.tile([P, nmmPh], f32)
        nc.sync.dma_start(pool_loc_h[:], plocH[:])
        pool_idx_t = const.tile([P, TPt // 16], i16)
        nc.sync.dma_start(pool_idx_t[:], psrcT[:])
        pool_loc_t = const.tile([P, nmmPt], f32)
        nc.sync.dma_start(pool_loc_t[:], plocT[:])

        # cross-hop accumulator: hacc = sum_k temp[k+1] * xnew_k  (b_sc and
        # temp[0]*embed are applied at zext build time)
        hacc = const.tile([P, NTILE, 2 * D], bf16)

        # precompute per-super last mm index (same schedule for all hops)
        last_mm_of_super = {}
        for j, (ch, b, s0, s1) in enumerate(mmH):
            last_mm_of_super[b // 2] = j

        nbatch = (TH + GB - 1) // GB
        PF = 3  # gather prefetch depth

        def emit_update(k):
            """hacc (+)= temp[k+1] * xnew_k, 4 tiles per DMA."""
            for t0 in range(0, NTILE, 4):
                nt = min(4, NTILE - t0)
                xn4 = sb.tile([P, 4, 2 * D], bf16, tag="xn4")
                nc.sync.dma_start(
                    xn4[:, 0:nt, :],
                    xnews[k][:].rearrange("(t p2 h) d -> p2 t (h d)",
                                          p2=P, h=2)[:, t0:t0 + nt, :])
                if k == 0:
                    nc.vector.tensor_scalar(
                        out=hacc[:, t0:t0 + nt, :], in0=xn4[:, 0:nt, :],
                        scalar1=temp_sb[:, k + 1:k + 2], scalar2=None,
                        op0=mybir.AluOpType.mult)
                else:
                    tmp = sb.tile([P, 4, 2 * D], bf16, tag="utmp")
                    nc.vector.tensor_scalar(
                        out=tmp[:, 0:nt, :], in0=xn4[:, 0:nt, :],
                        scalar1=temp_sb[:, k + 1:k + 2], scalar2=None,
                        op0=mybir.AluOpType.mult)
                    nc.vector.tensor_tensor(
                        out=hacc[:, t0:t0 + nt, :],
                        in0=hacc[:, t0:t0 + nt, :], in1=tmp[:, 0:nt, :],
                        op=mybir.AluOpType.add)

        # ---------------- zext = [z | 1 | s | 0...] per node ----------------
        # z = temp0*embed + b_sc * (hacc + temp3*xnew2); emitted per RS slice
        # inside hop 2's matmul stream (hop-2 update fused).
        def emit_zext(t0, nt):
            xn4 = sb.tile([P, 2, 2 * D], bf16, tag="xn4z")
            nc.sync.dma_start(
                xn4[:, 0:nt, :],
                xnews[KHOP - 1][:].rearrange("(t p2 h) d -> p2 t (h d)",
                                             p2=P, h=2)[:, t0:t0 + nt, :])
            emb4 = sb.tile([P, 2, 2 * D], bf16, tag="emb4")
            nc.sync.dma_start(
                emb4[:, 0:nt, :],
                embed_bf.rearrange("(t p2 h) d -> p2 t (h d)",
                                   p2=P, h=2)[:, t0:t0 + nt, :])
            acc4 = sb.tile([P, 2, 2 * D], f32, tag="acc4")
            nc.vector.tensor_scalar(
                out=acc4[:, 0:nt, :], in0=xn4[:, 0:nt, :],
                scalar1=temp_sb[:, KHOP:KHOP + 1], scalar2=None,
                op0=mybir.AluOpType.mult)
            nc.vector.tensor_tensor(
                out=acc4[:, 0:nt, :], in0=acc4[:, 0:nt, :],
                in1=hacc[:, t0:t0 + nt, :], op=mybir.AluOpType.add)
            ze4 = sb.tile([P, 2, 2, 2 * D], bf16, tag="ze4")
            z4 = sb.tile([P, 2, 2 * D], f32, tag="z4")
            for t in range(t0, t0 + nt):
                ti = t - t0
                for h in range(2):
                    # z = b_sc*acc + temp0*emb
                    nc.scalar.activation(
                        z4[:, ti, h * D:(h + 1) * D],
                        acc4[:, ti, h * D:(h + 1) * D],
                        mybir.ActivationFunctionType.Copy,
                        scale=bsc_sb[:, t, h:h + 1])
                emb_sc = sb.tile([P, 2 * D], f32, tag="embsc")
                nc.vector.tensor_scalar(
                    out=emb_sc[:], in0=emb4[:, ti, :],
                    scalar1=temp_sb[:, 0:1], scalar2=None,
                    op0=mybir.AluOpType.mult)
                nc.vector.tensor_tensor(
                    out=z4[:, ti, :], in0=z4[:, ti, :], in1=emb_sc[:],
                    op=mybir.AluOpType.add)
                prod = sb.tile([P, 2 * D], f32, tag="prod")
                nc.vector.tensor_tensor(out=prod[:], in0=z4[:, ti, :],
                                        in1=wrep[:], op=mybir.AluOpType.mult)
                s2 = sb.tile([P, 2, 1], f32, tag="s2")
                nc.vector.reduce_sum(
                    s2[:], prod[:].rearrange("p (h d) -> p h d", h=2),
                    axis=mybir.AxisListType.X)
                for h in range(2):
                    nc.scalar.activation(ze4[:, ti, h, 0:D],
                                         z4[:, ti, h * D:(h + 1) * D],
                                         mybir.ActivationFunctionType.Copy)
                nc.gpsimd.memset(ze4[:, ti, :, D:], 0.0)
                nc.gpsimd.memset(ze4[:, ti, :, D:D + 1], 1.0)
                nc.gpsimd.tensor_copy(ze4[:, ti, :, D + 1:D + 2], s2[:])
            nc.sync.dma_start(
                zext[:].rearrange("(t p2 h) d -> p2 t (h d)",
                                  p2=P, h=2)[:, t0:t0 + nt, :],
                ze4[:, 0:nt, :].rearrange("p n h d -> p n (h d)"))

        # ---------------- hops ----------------
        with tc.tile_pool(name="psh", bufs=2, space="PSUM") as psh:
            for k in range(KHOP):
                xsrc = embed_bf[:] if k == 0 else xnews[k - 1][:]
                wtile = hop_w0 if k == 0 else hop_w12
                xpart = xparts[k]

                gtiles = [None] * nbatch
                emitted = [0]

                def need_batch(bi, xsrc=xsrc, gtiles=gtiles, emitted=emitted):
                    while emitted[0] <= min(bi + PF, nbatch - 1):
                        b_ = emitted[0]
                        t0 = b_ * GB
                        n_ = min(GB, TH - t0)
                        gt = gpool.tile([P, GB // P, D], bf16, tag="gt",
                                        name="gt")
                        gtiles[b_] = gt
                        nc.gpsimd.dma_gather(
                            gt[:, 0:n_ // P, :], xsrc,
                            hop_idx[:, t0 // 16:(t0 + n_) // 16],
                            n_, n_, D, single_packet=False)
                        emitted[0] += 1

                ps = {}
                ev4 = None
                pending = []   # deferred RS slices
                ncoll = [0]    # colls emitted this hop
                zdone = [0]    # zext tiles emitted (hop 2 only)

                def emit_zext_ready(k=k, ncoll=ncoll, zdone=zdone):
                    """Emit zext for every tile whose RS slice completed at
                    least ~1 slice ago (lag hides the collective latency so
                    the SP-queue wait is already satisfied)."""
                    if k != KHOP - 1:
                        return
                    ready = SCUM[max(0, ncoll[0] - 2)]
                    while zdone[0] < ready:
                        t0 = zdone[0]
                        nt = min(2, ready - t0)
                        emit_zext(t0, nt)
                        zdone[0] += nt

                for j, (ch, b, st, sp) in enumerate(mmH):
                    if j == JU and k > 0:
                        emit_update(k - 1)
                    need_batch(ch // (GB // P))
                    oh_t = ohp.tile([P, P], bf16, tag="oh", bufs=8)
                    nc.vector.tensor_scalar(
                        out=oh_t[:], in0=iota_b[:],
                        scalar1=hop_loc[:, j:j + 1],
                        scalar2=wtile[:, j:j + 1],
                        op0=mybir.AluOpType.is_equal,
                        op1=mybir.AluOpType.mult)
                    sup, h = b // 2, b % 2
                    if sup not in ps and st:
                        ps[sup] = psh.tile([P, 2, D], f32, tag=f"ps{sup % 2}",
                                           name=f"psh_{sup % 2}")
                    nc.tensor.matmul(
                        ps[sup][:, h, :], lhsT=oh_t[:],
                        rhs=gtiles[ch // (GB // P)][:, ch % (GB // P), :],
                        start=st, stop=sp)
                    if sp and last_mm_of_super.get(sup) == j:
                        gi = sup % EG
                        if gi == 0:
                            ev4 = sb.tile([P, EG, 2 * D], bf16,
                                          tag="ev4")
                        nc.scalar.activation(
                            ev4[:, gi, :].rearrange("p (h d) -> p h d", h=2),
                            ps[sup][:],
                            mybir.ActivationFunctionType.Copy)
                        del ps[sup]
                        if gi == EG - 1:
                            g = sup // EG
                            nc.sync.dma_start(
                                xpart[:].rearrange("(g gi p2) d -> p2 g gi d",
                                                   gi=EG, p2=P)[:, g, :, :],
                                ev4[:])
                        if sup + 1 in SUPB:
                            sl = SUPB[sup + 1]
                            r0 = SUPCUM[sl] * P
                            r1 = SUPCUM[sl + 1] * P
                            n0 = SCUM[sl] * 256
                            n1 = SCUM[sl + 1] * 256
                            pending.append((xpart[:][r0:r1, :],
                                            xnews[k][:][n0:n1, :]))
                        if pending and sup in SUPTRIG:
                            coll(*pending.pop(0))
                            ncoll[0] += 1
                            emit_zext_ready()
                for args in pending:
                    coll(*args)
                    ncoll[0] += 1
                    emit_zext_ready()
            for t0 in range(zdone[0], NTILE, 2):
                emit_zext(t0, min(2, NTILE - t0))

        # ---------------- pooling (owner-sharded partials) ----------------
        def pool_stream(TP, idx_t, tag):
            """Emit gathers for one pool stream; return (gtiles, ecol)."""
            nbatch = (TP + GBP - 1) // GBP
            gtiles = []
            for bi in range(nbatch):
                t0 = bi * GBP
                n_ = min(GBP, TP - t0)
                gt = gppool.tile([P, GBP // P, 2 * D], bf16, tag="gtp" + tag,
                                 name="gt" + tag)
                gtiles.append(gt)
                nc.gpsimd.dma_gather(
                    gt[:, 0:n_ // P, :], zext[:],
                    idx_t[:, t0 // 16:(t0 + n_) // 16],
                    n_, n_, 2 * D, single_packet=False)
            nch = TP // P
            ecol = const.tile([P, nch], f32, tag="ecol" + tag, name="ecol" + tag)
            for bi in range(nbatch):
                c0 = bi * (GBP // P)
                nb_ = min(GBP // P, nch - c0)
                nc.scalar.activation(
                    ecol[:, c0:c0 + nb_].unsqueeze(2),
                    gtiles[bi][:, 0:nb_, D + 1:D + 2],
                    mybir.ActivationFunctionType.Exp)
            return gtiles, ecol

        def by_super(mmP):
            d = {}
            for j, (ch, b, st, sp) in enumerate(mmP):
                d.setdefault(b // 2, []).append((j, ch, b, st, sp))
            return d

        psm_holder = {}

        with tc.tile_pool(name="psp", bufs=2, space="PSUM") as psp:

            def emit_mlp(t):
                psm = psm_holder["psm"]
                """normalize + MLP for pooled-segment tile t (128 segs)."""
                if t % 4 == 0:
                    mlp_t["po4"] = sb.tile([P, 4, 260], bf16, tag="po4", name="po4")
                    nc.sync.dma_start(
                        mlp_t["po4"][:],
                        pout[:].rearrange("(t p) d -> p t d",
                                          p=P)[:, t:t + 4, :])
                    mlp_t["lt4"] = sb.tile([P, 4, R], f32, tag="lt4", name="lt4")
                po4 = mlp_t["po4"]
                lt4 = mlp_t["lt4"]
                fts = []
                for (c0, tagf) in ((0, "fh"), (130, "ft")):
                    den = sb.tile([P, 1], f32, tag="den" + tagf)
                    nc.vector.tensor_scalar(out=den[:],
                                            in0=po4[:, t % 4, c0 + D:c0 + D + 1],
                                            scalar1=1e-30, scalar2=None,
                                            op0=mybir.AluOpType.max)
                    rden = sb.tile([P, 1], f32, tag="rden" + tagf)
                    nc.vector.reciprocal(rden[:], den[:])
                    pooled = sb.tile([P, D], f32, tag="pl" + tagf)
                    nc.any.tensor_scalar(out=pooled[:],
                                         in0=po4[:, t % 4, c0:c0 + D],
                                         scalar1=rden[:], scalar2=None,
                                         op0=mybir.AluOpType.mult)
                    pt = psm.tile([P, P], f32, tag="pt")
                    nc.tensor.transpose(out=pt[:], in_=pooled[:],
                                        identity=ident[:])
                    ft = sb.tile([P, P], bf16, tag="ftr" + tagf)
                    nc.vector.tensor_copy(ft[:], pt[:])
                    fts.append(ft)
                htT = sb.tile([P, P], bf16, tag="htT")
                nc.any.tensor_tensor(out=htT[:], in0=fts[0][:], in1=fts[1][:],
                                     op=mybir.AluOpType.mult)
                feats = [fts[0], fts[1], htT]

                o1 = sb.tile([P, 4, P], bf16, tag="o1")
                for m in range(4):
                    ps1 = psm.tile([P, P], f32, tag="ps1")
                    for kk in range(3):
                        nc.tensor.matmul(ps1[:],
                                         lhsT=w1t[:, kk, m * P:(m + 1) * P],
                                         rhs=feats[kk][:],
                                         start=kk == 0, stop=kk == 2)
                    nc.scalar.activation(o1[:, m, :], ps1[:],
                                         mybir.ActivationFunctionType.Relu,
                                         bias=b1t[:, m:m + 1])
                ps2 = psm.tile([R, P], f32, tag="ps2", padded_shape=[P, P])
                for kk in range(4):
                    nc.tensor.matmul(ps2[:], lhsT=w2t[:, kk, :],
                                     rhs=o1[:, kk, :],
                                     start=kk == 0, stop=kk == 3)
                lg = sb.tile([R, P], f32, tag="lg")
                nc.vector.tensor_scalar(out=lg[:], in0=ps2[:], scalar1=b2t[:],
                                        scalar2=None, op0=mybir.AluOpType.add)
                lt = psm.tile([P, R], f32, tag="lt", padded_shape=[P, P])
                nc.tensor.transpose(out=lt[:], in_=lg[:], identity=ident[:R, :R])
                nc.vector.tensor_copy(lt4[:, t % 4, :], lt[:])
                if t % 4 == 3:
                    nc.sync.dma_start(
                        out.rearrange("(t p) r -> p t r",
                                      p=P)[:, t - 3:t + 1, :],
                        lt4[:])

            mlp_t = {}
            NSB = BSEG // P          # 32 segment tiles
            TPS = NSB // NSPLITP     # 8 tiles per pool RS slice
            mdone = [0]
            npcoll = [0]

            def emit_mlp_ready():
                ready = max(0, npcoll[0] - 2) * TPS
                while mdone[0] < ready:
                    emit_mlp(mdone[0])
                    mdone[0] += 1

            pending = []
            gtH, ecolH = pool_stream(TPh, pool_idx_h, "H")
            gtT, ecolT = pool_stream(TPt, pool_idx_t, "T")
            supH, supT = by_super(mmPh), by_super(mmPt)
            ev2 = None
            for sup in range(BSUP):
                gi = sup % 2
                if gi == 0:
                    ev2 = sb.tile([P, 4, 260], bf16, tag="evp")
                for (pn, sups, gts, ecol, loc_t, c0) in (
                        ("H", supH, gtH, ecolH, pool_loc_h, 0),
                        ("T", supT, gtT, ecolT, pool_loc_t, 130)):
                    pp = psp.tile([P, 2, 130], f32, tag=f"pp{pn}{sup % 2}",
                                  name=f"psp{pn}{sup % 2}")
                    touched = [False, False]
                    for (j, ch, b, st, sp) in sups.get(sup, []):
                        ohw_t = ohp.tile([P, P], bf16, tag="ohwp", bufs=8,
                                         name="ohw")
                        nc.vector.tensor_scalar(
                            out=ohw_t[:], in0=iota_b[:],
                            scalar1=loc_t[:, j:j + 1],
                            scalar2=ecol[:, ch:ch + 1],
                            op0=mybir.AluOpType.is_equal,
                            op1=mybir.AluOpType.mult)
                        h = b % 2
                        touched[h] = True
                        nc.tensor.matmul(
                            pp[:, h, :], lhsT=ohw_t[:],
                            rhs=gts[ch // (GBP // P)][:, ch % (GBP // P), 0:130],
                            start=st, stop=sp)
                    assert touched[0] and touched[1], "empty pool bucket"
                    nc.scalar.activation(
                        ev2[:, gi * 2:gi * 2 + 2, c0:c0 + 130],
                        pp[:],
                        mybir.ActivationFunctionType.Copy)
                if gi == 1:
                    g = sup // 2
                    nc.sync.dma_start(
                        ppart[:].rearrange("(g q p2) d -> p2 g q d",
                                           q=4, p2=P)[:, g, :, :],
                        ev2[:])
                if sup + 1 in PSUPB:
                    sl = PSUPB[sup + 1]
                    r0 = PSUPCUM[sl] * 2 * P
                    r1 = PSUPCUM[sl + 1] * 2 * P
                    n0 = PCUM[sl] * 256
                    n1 = PCUM[sl + 1] * 256
                    pending.append((ppart[:][r0:r1, :], pout[:][n0:n1, :]))
                if pending and sup in PSUPTRIG:
                    coll(*pending.pop(0))
            for args in pending:
                coll(*args)

        # ---------------- normalize + MLP (my 4096 segments) ----------------
        with tc.tile_pool(name="psm", bufs=2, space="PSUM") as psm:
            psm_holder["psm"] = psm
            for t in range(NSB):
                emit_mlp(t)

    nc.compile()
    return nc


def kernel(embed, temp, attn_w, attn_b, W1, b1, W2, b2,
           edge_index, H_idx, H_seg, T_idx, T_seg, B):
    embed = np.asarray(embed, np.float32)
    temp = np.asarray(temp, np.float32)
    attn_w = np.asarray(attn_w, np.float32)
    W1 = np.asarray(W1, np.float32)
    b1 = np.asarray(b1, np.float32)
    W2 = np.asarray(W2, np.float32)
    b2 = np.asarray(b2, np.float32)
    edge_index = np.asarray(edge_index)
    H_idx, H_seg = np.asarray(H_idx), np.asarray(H_seg)
    T_idx, T_seg = np.asarray(T_idx), np.asarray(T_seg)

    S = _make_schedules(edge_index, H_idx, H_seg, T_idx, T_seg)
    TH, mmH = S["TH"], S["mmH"]
    TPh, mmPh, TPt, mmPt = S["TPh"], S["mmPh"], S["TPt"], S["mmPt"]

    key = (TH, len(mmH), TPh, len(mmPh), TPt, len(mmPt))
    if key not in _COMPILED:
        _COMPILED[key] = _build_program(TH, mmH, len(mmPh), mmPh,
                                        len(mmPt), mmPt, TPh, TPt)
    nc = _COMPILED[key]

    def pair_layout_f32(v, c):
        # [p, t, h] for nodes (t*128+p)*2+h of core c
        lo = c * SHARD
        arr = v[lo:lo + SHARD].reshape(NTILE, P, 2)
        return np.ascontiguousarray(arr.transpose(1, 0, 2).astype(np.float32))

    bf = ml_dtypes.bfloat16
    in_maps = []
    for c in range(NCORES):
        lo = c * SHARD
        n_real = max(0, min(SHARD, N - lo))
        esh = np.zeros((SHARD, D), np.float32)
        esh[:n_real] = embed[lo:lo + n_real]
        wr = np.tile(attn_w[:, 0][None, :], (P, 2))
        in_maps.append(dict(
            embed_bf=esh.astype(bf),
            bsc_in=pair_layout_f32(S["b_sc"], c),
            temp_in=np.tile(temp[None, :], (P, 1)),
            wrep_in=wr,
            w1_in=W1.reshape(3, P, H_MLP).astype(bf),
            b1_in=np.ascontiguousarray(b1.reshape(4, P).T),
            w2_in=W2.reshape(4, P, R).astype(bf),
            b2_in=b2[:, None].copy(),
            hsrc=_wrap_idx16(S["hidxs"][c]),
            hloc=S["hlocs"][c],
            hw0=S["hw0s"][c],
            hw12=S["hw12s"][c],
            psrcH=_wrap_idx16(S["pidxsH"][c]),
            plocH=S["plocsH"][c],
            psrcT=_wrap_idx16(S["pidxsT"][c]),
            plocT=S["plocsT"][c],
        ))

    res = run_bass_kernel_spmd(nc, in_maps, list(range(NCORES)))
    return np.concatenate([res.results[c]["out"] for c in range(NCORES)], axis=0)


# revision 29
# speedup vs baseline: 1.0215x; 1.0215x over previous
import sys

sys.path.insert(0, "/opt/trn_rl_repo")

from contextlib import ExitStack

import numpy as np
import ml_dtypes
import concourse.bacc as bacc
import concourse.mybir as mybir
from concourse.bass_utils import run_bass_kernel_spmd
from concourse.tile import TileContext
from concourse.masks import make_identity

P = 128
NCORES = 8
N, D, E, KHOP, B, L = 100000, 128, 1600000, 3, 32768, 262144
H_MLP, R = 512, 64
SHARD = 12544           # nodes per core (98 * 128); core 7 padded
NP = NCORES * SHARD     # 100352
NPAIR = SHARD // 2      # 6272 pair-rows per core
NSUP = NP // 256        # 392 dst superblocks (256 nodes each)
BSEG = B // NCORES      # 4096 segments per core
BSUP = B // 256         # 128 seg superblocks
NTILE = SHARD // 256    # 49 pair tiles per core
NSUPO = NSUP // NCORES  # 49 dst superblocks per owner
SLICES = [7, 7, 7, 7, 7, 7, 7]   # hop RS slices, per-owner supers (sum 49)
NSPLIT = len(SLICES)
SCUM = [0]
for _s in SLICES:
    SCUM.append(SCUM[-1] + _s)       # per-owner cumulative
SUPCUM = [c * NCORES for c in SCUM]  # global super boundaries
PSLICES = [4, 4, 4, 4]               # pool RS slices, per-owner supers (sum 16)
NSPLITP = len(PSLICES)
PCUM = [0]
for _s in PSLICES:
    PCUM.append(PCUM[-1] + _s)
PSUPCUM = [c * NCORES for c in PCUM]
SUPB = {SUPCUM[j + 1]: j for j in range(NSPLIT)}       # slice end -> idx
SUPTRIG = {min(SUPCUM[j] + 28, SUPCUM[j + 1]) - 1: j   # deferred-coll point
           for j in range(1, NSPLIT)}
PSUPB = {PSUPCUM[j + 1]: j for j in range(NSPLITP)}
PSUPTRIG = {min(PSUPCUM[j] + 20, PSUPCUM[j + 1]) - 1: j
            for j in range(1, NSPLITP)}
QHOP = 4                # gsz quantum for hop schedule
QPOOL = 16              # gsz quantum for pool schedule
GB = 3072               # gather batch tokens (hops)
GBP = 1024              # gather batch tokens (pools)
EG = 4                  # supers per evac DMA group
JU = 256                # mm index at which prior hop's update is emitted

f32 = mybir.dt.float32
bf16 = mybir.dt.bfloat16
i16 = mybir.dt.int16
i32 = mybir.dt.int32

_COMPILED = {}


def _wrap_idx16(idx):
    """dma_gather index layout: token i -> partition i%16, col i//16, x8 replicated."""
    n = len(idx)
    assert n % 16 == 0
    return np.tile(idx.reshape(n // 16, 16).T.astype(np.int16), (8, 1))


def _schedule(bucket_by_core, slot_by_core, idx_by_core, w_by_core,
              nbuckets, quantum):
    """Static SPMD schedule for one-hot scatter matmuls.

    Tokens are sorted by bucket; each bucket's region is padded to `quantum`
    (shared across cores via max count). Chunks of 128 tokens may straddle
    bucket boundaries; straddling chunks get one matmul per bucket touched,
    with masked loc columns.

    Returns (T, mm, idx_streams, loc_tables, w_tables):
      T: padded token count (mult of 128)
      mm: list of (chunk, bucket, start, stop)
      idx_streams[c]: int16 [T] gather indices (pad -> 0)
      loc_tables[c]: f32 [128, nmm] per-matmul slot columns (pad/mask -> -1)
      w_tables[c]: f32 [128, nmm] per-matmul token weights (or None)
    """
    ncores = len(bucket_by_core)
    counts = np.zeros((ncores, nbuckets), np.int64)
    for c in range(ncores):
        np.add.at(counts[c], bucket_by_core[c], 1)
    gsz = ((counts.max(axis=0) + quantum - 1) // quantum) * quantum
    # every bucket must get at least one matmul so its psum slice is
    # written (zeros) before the evacuation reads it
    gsz[gsz == 0] = quantum
    starts = np.zeros(nbuckets + 1, np.int64)
    starts[1:] = np.cumsum(gsz)
    T = int(((starts[-1] + P - 1) // P) * P)

    # static matmul descriptors
    mm = []
    for b in range(nbuckets):
        c0 = int(starts[b]) // P
        c1 = int(starts[b + 1] - 1) // P
        for ch in range(c0, c1 + 1):
            mm.append([ch, b, ch == c0, ch == c1])
    nmm = len(mm)

    idx_streams, loc_tables, w_tables = [], [], []
    for c in range(ncores):
        order = np.argsort(bucket_by_core[c], kind="stable")
        bs = bucket_by_core[c][order]
        sl = slot_by_core[c][order].astype(np.float32)
        ix = idx_by_core[c][order].astype(np.int16)
        run_start = np.concatenate([[0], np.cumsum(counts[c])])
        pos_in_run = np.arange(len(bs)) - run_start[bs]
        out_pos = starts[bs] + pos_in_run
        idx_full = np.zeros(T, np.int16)
        idx_full[out_pos] = ix
        slot_full = np.full(T, -1.0, np.float32)
        slot_full[out_pos] = sl
        bkt_full = np.full(T, -1, np.int64)
        bkt_full[out_pos] = bs
        w_full = np.zeros(T, np.float32)
        if w_by_core is not None:
            w_full[out_pos] = w_by_core[c][order].astype(np.float32)
        # build per-mm loc columns: tokens of chunk ch masked to bucket b
        loc_cols = np.full((nmm, P), -1.0, np.float32)
        w_cols = np.zeros((nmm, P), np.float32)
        for j, (ch, b, _s, _e) in enumerate(mm):
            tok = slice(ch * P, ch * P + P)
            m = bkt_full[tok] == b
            col = np.full(P, -1.0, np.float32)
            col[m] = slot_full[tok][m]
            loc_cols[j] = col
            w_cols[j] = w_full[tok]
        idx_streams.append(idx_full)
        loc_tables.append(np.ascontiguousarray(loc_cols.T))  # [128, nmm]
        w_tables.append(np.ascontiguousarray(w_cols.T))
    return T, mm, idx_streams, loc_tables, w_tables


def _slice_rank(nsup, npo, cum, supcum):
    """Super processing order (slice-major, owner-interleaved) so RS slice j
    covers supers [supcum[j], supcum[j+1]) = every owner's cum[j]..cum[j+1]
    supers, and RS output rows land in natural per-owner order."""
    s = np.arange(nsup)
    o, w = s // npo, s % npo
    j = np.searchsorted(np.asarray(cum), w, side="right") - 1
    pos = w - np.asarray(cum)[j]
    sz = np.asarray(cum)[j + 1] - np.asarray(cum)[j]
    return np.asarray(supcum)[j] + o * sz + pos


def _rank_of_sup():
    return _slice_rank(NSUP, NSUPO, SCUM, SUPCUM)


def _rank_of_psup():
    return _slice_rank(BSUP, BSUP // NCORES, PCUM, PSUPCUM)


def _make_schedules(edge_index, H_idx, H_seg, T_idx, T_seg):
    src = edge_index[0].astype(np.int64)
    dst = edge_index[1].astype(np.int64)

    deg_dst = np.bincount(dst, minlength=NP)
    deg_src = np.bincount(src, minlength=NP)
    a_sc = 1.0 / np.sqrt(np.maximum(deg_src, 1.0))
    b_sc = 1.0 / np.sqrt(np.maximum(deg_dst, 1.0))
    absc = (a_sc * b_sc).astype(np.float32)
    a_sc = a_sc.astype(np.float32)

    rank = _rank_of_sup()
    hb, hs, hi, hw0, hw12 = [], [], [], [], []
    for c in range(NCORES):
        lo = c * SHARD
        m = (src >= lo) & (src < lo + SHARD)
        d = dst[m]
        s_ = src[m]
        hb.append((rank[d >> 8] * 2 + (d & 1)).astype(np.int64))
        hs.append((d >> 1) & 127)
        hi.append(s_ - lo)
        hw0.append(a_sc[s_])
        hw12.append(absc[s_])
    TH, mmH, hidxs, hlocs, hw0s = _schedule(hb, hs, hi, hw0, NSUP * 2, QHOP)
    _, _, _, _, hw12s = _schedule(hb, hs, hi, hw12, NSUP * 2, QHOP)

    prank = _rank_of_psup()

    def pool_sched(idx, seg):
        pb, psl, pii = [], [], []
        idx = idx.astype(np.int64)
        seg = seg.astype(np.int64)
        for c in range(NCORES):
            lo = c * SHARD
            m = (idx >= lo) & (idx < lo + SHARD)
            sg = seg[m]
            pb.append((prank[sg >> 8] * 2 + ((sg >> 7) & 1)).astype(np.int64))
            psl.append(sg & 127)
            pii.append(idx[m] - lo)
        return _schedule(pb, psl, pii, None, BSUP * 2, QPOOL)

    TPh, mmPh, pidxsH, plocsH, _ = pool_sched(H_idx, H_seg)
    TPt, mmPt, pidxsT, plocsT, _ = pool_sched(T_idx, T_seg)
    return dict(TH=TH, mmH=mmH, hidxs=hidxs, hlocs=hlocs, hw0s=hw0s,
                hw12s=hw12s, TPh=TPh, mmPh=mmPh, pidxsH=pidxsH,
                plocsH=plocsH, TPt=TPt, mmPt=mmPt, pidxsT=pidxsT,
                plocsT=plocsT, b_sc=b_sc.astype(np.float32))


def _build_program(TH, mmH, nmmPh, mmPh, nmmPt, mmPt, TPh, TPt):
    nc = bacc.Bacc("TRN2", target_bir_lowering=False, num_devices=NCORES)

    nmmH = len(mmH)

    # ---------------- dram tensors ----------------
    embed_bf = nc.dram_tensor("embed_bf", [SHARD, D], bf16, kind="ExternalInput")
    bsc_in = nc.dram_tensor("bsc_in", [P, NTILE, 2], f32, kind="ExternalInput")
    temp_in = nc.dram_tensor("temp_in", [P, 4], f32, kind="ExternalInput")
    wrep_in = nc.dram_tensor("wrep_in", [P, 2 * D], f32, kind="ExternalInput")
    w1_in = nc.dram_tensor("w1_in", [3, P, H_MLP], bf16, kind="ExternalInput")
    b1_in = nc.dram_tensor("b1_in", [P, 4], f32, kind="ExternalInput")
    w2_in = nc.dram_tensor("w2_in", [4, P, R], bf16, kind="ExternalInput")
    b2_in = nc.dram_tensor("b2_in", [R, 1], f32, kind="ExternalInput")
    hsrc = nc.dram_tensor("hsrc", [P, TH // 16], i16, kind="ExternalInput")
    hloc = nc.dram_tensor("hloc", [P, nmmH], f32, kind="ExternalInput")
    hw0_in = nc.dram_tensor("hw0", [P, nmmH], f32, kind="ExternalInput")
    hw12_in = nc.dram_tensor("hw12", [P, nmmH], f32, kind="ExternalInput")
    psrcH = nc.dram_tensor("psrcH", [P, TPh // 16], i16, kind="ExternalInput")
    plocH = nc.dram_tensor("plocH", [P, nmmPh], f32, kind="ExternalInput")
    psrcT = nc.dram_tensor("psrcT", [P, TPt // 16], i16, kind="ExternalInput")
    plocT = nc.dram_tensor("plocT", [P, nmmPt], f32, kind="ExternalInput")

    out = nc.dram_tensor("out", [BSEG, R], f32, kind="ExternalOutput")

    rg = [list(range(NCORES))]

    with TileContext(nc) as tc, ExitStack() as ctx:
        sb = ctx.enter_context(tc.tile_pool(name="sb", bufs=2))
        dram = ctx.enter_context(tc.tile_pool(name="dram", bufs=1,
                                              space="DRAM"))
        xparts = [dram.tile([NP // 2, 2 * D], bf16, tag=f"xpart{k}",
                            name=f"xpart{k}") for k in range(KHOP)]
        xnews = [dram.tile([SHARD, D], bf16, tag=f"xnew{k}",
                           name=f"xnew{k}") for k in range(KHOP)]
        zext = dram.tile([SHARD, 2 * D], bf16, tag="zext", name="zext")
        ppart = dram.tile([B, 260], bf16, tag="ppart", name="ppart")
        pout = dram.tile([BSEG, 260], bf16, tag="pout", name="pout")
        const = ctx.enter_context(tc.tile_pool(name="const", bufs=1))
        gpool = ctx.enter_context(tc.tile_pool(name="gath", bufs=4))
        gppool = ctx.enter_context(tc.tile_pool(name="gathp", bufs=2))
        ohp = ctx.enter_context(tc.tile_pool(name="ohp", bufs=2))
        def coll(ins_ap, outs_ap):
            """Async ReduceScatter; DRAM-tile dependency tracking orders it
            after the producing DMAs and before consumers."""
            nc.gpsimd.collective_compute(
                "ReduceScatter", mybir.AluOpType.add,
                ins=[ins_ap], outs=[outs_ap], replica_groups=rg,
            )

        def gather_fence(dram_tile, nrows):
            """dma_gather's DRAM source AP bypasses the tile dependency
            tracker (custom BIR DMA lowering), so gathers can race the
            writes that produce their source. Fence via tracked standard
            ops only: an SP dma_start probe reads one row of every
            256-row block (ordered after all writers by the tracker), and
            a Pool-engine copy of the probe tile fences the in-order Pool
            queue - gathers emitted later cannot start earlier."""
            probe = sb.tile([P, D], bf16, tag="probe", name="probe")
            nc.sync.dma_start(
                probe[:],
                dram_tile[:].rearrange("(t r) d -> t r d",
                                       r=nrows // P)[:, 0, 0:D])
            pcp = sb.tile([P, D], bf16, tag="pcp", name="pcp")
            nc.gpsimd.tensor_copy(pcp[:], probe[:])

        # hop gather index/loc/weight tables first: hop-0 work depends on them
        hop_idx = const.tile([P, TH // 16], i16)
        nc.sync.dma_start(hop_idx[:], hsrc[:])
        hop_loc = const.tile([P, nmmH], f32)
        nc.sync.dma_start(hop_loc[:], hloc[:])
        hop_w0 = const.tile([P, nmmH], f32)
        nc.sync.dma_start(hop_w0[:], hw0_in[:])
        hop_w12 = const.tile([P, nmmH], f32)
        nc.sync.dma_start(hop_w12[:], hw12_in[:])

        # ---------------- constants ----------------
        iota_i = const.tile([P, P], i32)
        nc.gpsimd.iota(iota_i[:], pattern=[[1, P]], base=0, channel_multiplier=0)
        iota_b = const.tile([P, P], bf16)
        nc.vector.tensor_copy(iota_b[:], iota_i[:])
        ident = const.tile([P, P], f32)
        make_identity(nc, ident[:])

        temp_sb = const.tile([P, 4], f32)
        nc.sync.dma_start(temp_sb[:], temp_in[:])
        bsc_sb = const.tile([P, NTILE, 2], f32)
        nc.sync.dma_start(bsc_sb[:], bsc_in[:])
        wrep = const.tile([P, 2 * D], f32)
        nc.sync.dma_start(wrep[:], wrep_in[:])
        w1t = const.tile([P, 3, H_MLP], bf16)
        nc.sync.dma_start(w1t[:], w1_in.rearrange("k p h -> p k h")[:])
        b1t = const.tile([P, 4], f32)
        nc.sync.dma_start(b1t[:], b1_in[:])
        w2t = const.tile([P, 4, R], bf16)
        nc.sync.dma_start(w2t[:], w2_in.rearrange("k p r -> p k r")[:])
        b2t = const.tile([R, 1], f32)
        nc.sync.dma_start(b2t[:], b2_in[:])

        # pool tables (needed much later; loaded early, DMA is cheap)
        pool_idx_h = const.tile([P, TPh // 16], i16)
        nc.sync.dma_start(pool_idx_h[:], psrcH[:])
        pool_loc_h = const.tile([P, nmmPh], f32)
        nc.sync.dma_start(pool_loc_h[:], plocH[:])
        pool_idx_t = const.tile([P, TPt // 16], i16)
        nc.sync.dma_start(pool_idx_t[:], psrcT[:])
        pool_loc_t = const.tile([P, nmmPt], f32)
        nc.sync.dma_start(pool_loc_t[:], plocT[:])

        # cross-hop accumulator: hacc = sum_k temp[k+1] * xnew_k  (b_sc and
        # temp[0]*embed are applied at zext build time)
        hacc = const.tile([P, NTILE, 2 * D], bf16)

        # precompute per-super last mm index (same schedule for all hops)
        last_mm_of_super = {}
        for j, (ch, b, s0, s1) in enumerate(mmH):
            last_mm_of_super[b // 2] = j

        nbatch = (TH + GB - 1) // GB
        PF = 3  # gather prefetch depth

        def emit_update(k):
            """hacc (+)= temp[k+1] * xnew_k, 4 tiles per DMA."""
            for t0 in range(0, NTILE, 4):
                nt = min(4, NTILE - t0)
                xn4 = sb.tile([P, 4, 2 * D], bf16, tag="xn4")
                nc.sync.dma_start(
                    xn4[:, 0:nt, :],
                    xnews[k][:].rearrange("(t p2 h) d -> p2 t (h d)",
                                          p2=P, h=2)[:, t0:t0 + nt, :])
                if k == 0:
                    nc.vector.tensor_scalar(
                        out=hacc[:, t0:t0 + nt, :], in0=xn4[:, 0:nt, :],
                        scalar1=temp_sb[:, k + 1:k + 2], scalar2=None,
                        op0=mybir.AluOpType.mult)
                else:
                    tmp = sb.tile([P, 4, 2 * D], bf16, tag="utmp")
                    nc.vector.tensor_scalar(
                        out=tmp[:, 0:nt, :], in0=xn4[:, 0:nt, :],
                        scalar1=temp_sb[:, k + 1:k + 2], scalar2=None,
                        op0=mybir.AluOpType.mult)
                    nc.vector.tensor_tensor(
                        out=hacc[:, t0:t0 + nt, :],
                        in0=hacc[:, t0:t0 + nt, :], in1=tmp[:, 0:nt, :],
                        op=mybir.AluOpType.add)

        # ---------------- zext = [z | 1 | s | 0...] per node ----------------
        # z = temp0*embed + b_sc * (hacc + temp3*xnew2); emitted per RS slice
        # inside hop 2's matmul stream (hop-2 update fused).
        def emit_zext(t0, nt):
            xn4 = sb.tile([P, 2, 2 * D], bf16, tag="xn4z")
            nc.sync.dma_start(
                xn4[:, 0:nt, :],
                xnews[KHOP - 1][:].rearrange("(t p2 h) d -> p2 t (h d)",
                                             p2=P, h=2)[:, t0:t0 + nt, :])
            emb4 = sb.tile([P, 2, 2 * D], bf16, tag="emb4")
            nc.sync.dma_start(
                emb4[:, 0:nt, :],
                embed_bf.rearrange("(t p2 h) d -> p2 t (h d)",
                                   p2=P, h=2)[:, t0:t0 + nt, :])
            acc4 = sb.tile([P, 2, 2 * D], f32, tag="acc4")
            nc.vector.tensor_scalar(
                out=acc4[:, 0:nt, :], in0=xn4[:, 0:nt, :],
                scalar1=temp_sb[:, KHOP:KHOP + 1], scalar2=None,
                op0=mybir.AluOpType.mult)
            nc.vector.tensor_tensor(
                out=acc4[:, 0:nt, :], in0=acc4[:, 0:nt, :],
                in1=hacc[:, t0:t0 + nt, :], op=mybir.AluOpType.add)
            ze4 = sb.tile([P, 2, 2, 2 * D], bf16, tag="ze4")
            z4 = sb.tile([P, 2, 2 * D], f32, tag="z4")
            for t in range(t0, t0 + nt):
                ti = t - t0
                for h in range(2):
                    # z = b_sc*acc + temp0*emb
                    nc.scalar.activation(
                        z4[:, ti, h * D:(h + 1) * D],
                        acc4[:, ti, h * D:(h + 1) * D],
                        mybir.ActivationFunctionType.Copy,
                        scale=bsc_sb[:, t, h:h + 1])
                emb_sc = sb.tile([P, 2 * D], f32, tag="embsc")
                nc.vector.tensor_scalar(
                    out=emb_sc[:], in0=emb4[:, ti, :],
                    scalar1=temp_sb[:, 0:1], scalar2=None,
                    op0=mybir.AluOpType.mult)
                nc.vector.tensor_tensor(
                    out=z4[:, ti, :], in0=z4[:, ti, :], in1=emb_sc[:],
                    op=mybir.AluOpType.add)
                prod = sb.tile([P, 2 * D], f32, tag="prod")
                nc.vector.tensor_tensor(out=prod[:], in0=z4[:, ti, :],
                                        in1=wrep[:], op=mybir.AluOpType.mult)
                s2 = sb.tile([P, 2, 1], f32, tag="s2")
                nc.vector.reduce_sum(
                    s2[:], prod[:].rearrange("p (h d) -> p h d", h=2),
                    axis=mybir.AxisListType.X)
                for h in range(2):
                    nc.scalar.activation(ze4[:, ti, h, 0:D],
                                         z4[:, ti, h * D:(h + 1) * D],
                                         mybir.ActivationFunctionType.Copy)
                nc.gpsimd.memset(ze4[:, ti, :, D:], 0.0)
                nc.gpsimd.memset(ze4[:, ti, :, D:D + 1], 1.0)
                nc.gpsimd.tensor_copy(ze4[:, ti, :, D + 1:D + 2], s2[:])
            nc.sync.dma_start(
                zext[:].rearrange("(t p2 h) d -> p2 t (h d)",
                                  p2=P, h=2)[:, t0:t0 + nt, :],
                ze4[:, 0:nt, :].rearrange("p n h d -> p n (h d)"))

        # ---------------- hops ----------------
        with tc.tile_pool(name="psh", bufs=2, space="PSUM") as psh:
            for k in range(KHOP):
                xsrc = embed_bf[:] if k == 0 else xnews[k - 1][:]
                wtile = hop_w0 if k == 0 else hop_w12
                xpart = xparts[k]

                gtiles = [None] * nbatch
                emitted = [0]

                def need_batch(bi, xsrc=xsrc, gtiles=gtiles, emitted=emitted):
                    while emitted[0] <= min(bi + PF, nbatch - 1):
                        b_ = emitted[0]
                        t0 = b_ * GB
                        n_ = min(GB, TH - t0)
                        gt = gpool.tile([P, GB // P, D], bf16, tag="gt",
                                        name="gt")
                        gtiles[b_] = gt
                        nc.gpsimd.dma_gather(
                            gt[:, 0:n_ // P, :], xsrc,
                            hop_idx[:, t0 // 16:(t0 + n_) // 16],
                            n_, n_, D, single_packet=False)
                        emitted[0] += 1

                if k > 0:
                    gather_fence(xnews[k - 1], SHARD)
                ps = {}
                ev4 = None
                pending = []   # deferred RS slices
                ncoll = [0]    # colls emitted this hop
                zdone = [0]    # zext tiles emitted (hop 2 only)

                def emit_zext_ready(k=k, ncoll=ncoll, zdone=zdone):
                    """Emit zext for every tile whose RS slice completed at
                    least ~1 slice ago (lag hides the collective latency so
                    the SP-queue wait is already satisfied)."""
                    if k != KHOP - 1:
                        return
                    ready = SCUM[max(0, ncoll[0] - 2)]
                    while zdone[0] < ready:
                        t0 = zdone[0]
                        nt = min(2, ready - t0)
                        emit_zext(t0, nt)
                        zdone[0] += nt

                for j, (ch, b, st, sp) in enumerate(mmH):
                    if j == JU and k > 0:
                        emit_update(k - 1)
                    need_batch(ch // (GB // P))
                    oh_t = ohp.tile([P, P], bf16, tag="oh", bufs=8)
                    nc.vector.tensor_scalar(
                        out=oh_t[:], in0=iota_b[:],
                        scalar1=hop_loc[:, j:j + 1],
                        scalar2=wtile[:, j:j + 1],
                        op0=mybir.AluOpType.is_equal,
                        op1=mybir.AluOpType.mult)
                    sup, h = b // 2, b % 2
                    if sup not in ps and st:
                        ps[sup] = psh.tile([P, 2, D], f32, tag=f"ps{sup % 2}",
                                           name=f"psh_{sup % 2}")
                    nc.tensor.matmul(
                        ps[sup][:, h, :], lhsT=oh_t[:],
                        rhs=gtiles[ch // (GB // P)][:, ch % (GB // P), :],
                        start=st, stop=sp)
                    if sp and last_mm_of_super.get(sup) == j:
                        gi = sup % EG
                        if gi == 0:
                            ev4 = sb.tile([P, EG, 2 * D], bf16,
                                          tag="ev4")
                        nc.scalar.activation(
                            ev4[:, gi, :].rearrange("p (h d) -> p h d", h=2),
                            ps[sup][:],
                            mybir.ActivationFunctionType.Copy)
                        del ps[sup]
                        if gi == EG - 1:
                            g = sup // EG
                            nc.sync.dma_start(
                                xpart[:].rearrange("(g gi p2) d -> p2 g gi d",
                                                   gi=EG, p2=P)[:, g, :, :],
                                ev4[:])
                        if sup + 1 in SUPB:
                            sl = SUPB[sup + 1]
                            r0 = SUPCUM[sl] * P
                            r1 = SUPCUM[sl + 1] * P
                            n0 = SCUM[sl] * 256
                            n1 = SCUM[sl + 1] * 256
                            pending.append((xpart[:][r0:r1, :],
                                            xnews[k][:][n0:n1, :]))
                        if pending and sup in SUPTRIG:
                            coll(*pending.pop(0))
                            ncoll[0] += 1
                            emit_zext_ready()
                for args in pending:
                    coll(*args)
                    ncoll[0] += 1
                    emit_zext_ready()
            for t0 in range(zdone[0], NTILE, 2):
                emit_zext(t0, min(2, NTILE - t0))

        # ---------------- pooling (owner-sharded partials) ----------------
        def pool_stream(TP, idx_t, tag):
            """Emit gathers for one pool stream; return (gtiles, ecol)."""
            nbatch = (TP + GBP - 1) // GBP
            gtiles = []
            for bi in range(nbatch):
                t0 = bi * GBP
                n_ = min(GBP, TP - t0)
                gt = gppool.tile([P, GBP // P, 2 * D], bf16, tag="gtp" + tag,
                                 name="gt" + tag)
                gtiles.append(gt)
                nc.gpsimd.dma_gather(
                    gt[:, 0:n_ // P, :], zext[:],
                    idx_t[:, t0 // 16:(t0 + n_) // 16],
                    n_, n_, 2 * D, single_packet=False)
            nch = TP // P
            ecol = const.tile([P, nch], f32, tag="ecol" + tag, name="ecol" + tag)
            for bi in range(nbatch):
                c0 = bi * (GBP // P)
                nb_ = min(GBP // P, nch - c0)
                nc.scalar.activation(
                    ecol[:, c0:c0 + nb_].unsqueeze(2),
                    gtiles[bi][:, 0:nb_, D + 1:D + 2],
                    mybir.ActivationFunctionType.Exp)
            return gtiles, ecol

        def by_super(mmP):
            d = {}
            for j, (ch, b, st, sp) in enumerate(mmP):
                d.setdefault(b // 2, []).append((j, ch, b, st, sp))
            return d

        psm_holder = {}

        with tc.tile_pool(name="psp", bufs=2, space="PSUM") as psp:

            def emit_mlp(t):
                psm = psm_holder["psm"]
                """normalize + MLP for pooled-segment tile t (128 segs)."""
                if t % 4 == 0:
                    mlp_t["po4"] = sb.tile([P, 4, 260], bf16, tag="po4", name="po4")
                    nc.sync.dma_start(
                        mlp_t["po4"][:],
                        pout[:].rearrange("(t p) d -> p t d",
                                          p=P)[:, t:t + 4, :])
                    mlp_t["lt4"] = sb.tile([P, 4, R], f32, tag="lt4", name="lt4")
                po4 = mlp_t["po4"]
                lt4 = mlp_t["lt4"]
                fts = []
                for (c0, tagf) in ((0, "fh"), (130, "ft")):
                    den = sb.tile([P, 1], f32, tag="den" + tagf)
                    nc.vector.tensor_scalar(out=den[:],
                                            in0=po4[:, t % 4, c0 + D:c0 + D + 1],
                                            scalar1=1e-30, scalar2=None,
                                            op0=mybir.AluOpType.max)
                    rden = sb.tile([P, 1], f32, tag="rden" + tagf)
                    nc.vector.reciprocal(rden[:], den[:])
                    pooled = sb.tile([P, D], f32, tag="pl" + tagf)
                    nc.any.tensor_scalar(out=pooled[:],
                                         in0=po4[:, t % 4, c0:c0 + D],
                                         scalar1=rden[:], scalar2=None,
                                         op0=mybir.AluOpType.mult)
                    pt = psm.tile([P, P], f32, tag="pt")
                    nc.tensor.transpose(out=pt[:], in_=pooled[:],
                                        identity=ident[:])
                    ft = sb.tile([P, P], bf16, tag="ftr" + tagf)
                    nc.vector.tensor_copy(ft[:], pt[:])
                    fts.append(ft)
                htT = sb.tile([P, P], bf16, tag="htT")
                nc.any.tensor_tensor(out=htT[:], in0=fts[0][:], in1=fts[1][:],
                                     op=mybir.AluOpType.mult)
                feats = [fts[0], fts[1], htT]

                o1 = sb.tile([P, 4, P], bf16, tag="o1")
                for m in range(4):
                    ps1 = psm.tile([P, P], f32, tag="ps1")
                    for kk in range(3):
                        nc.tensor.matmul(ps1[:],
                                         lhsT=w1t[:, kk, m * P:(m + 1) * P],
                                         rhs=feats[kk][:],
                                         start=kk == 0, stop=kk == 2)
                    nc.scalar.activation(o1[:, m, :], ps1[:],
                                         mybir.ActivationFunctionType.Relu,
                                         bias=b1t[:, m:m + 1])
                ps2 = psm.tile([R, P], f32, tag="ps2", padded_shape=[P, P])
                for kk in range(4):
                    nc.tensor.matmul(ps2[:], lhsT=w2t[:, kk, :],
                                     rhs=o1[:, kk, :],
                                     start=kk == 0, stop=kk == 3)
                lg = sb.tile([R, P], f32, tag="lg")
                nc.vector.tensor_scalar(out=lg[:], in0=ps2[:], scalar1=b2t[:],
                                        scalar2=None, op0=mybir.AluOpType.add)
                lt = psm.tile([P, R], f32, tag="lt", padded_shape=[P, P])
                nc.tensor.transpose(out=lt[:], in_=lg[:], identity=ident[:R, :R])
                nc.vector.tensor_copy(lt4[:, t % 4, :], lt[:])
                if t % 4 == 3:
                    nc.sync.dma_start(
                        out.rearrange("(t p) r -> p t r",
                                      p=P)[:, t - 3:t + 1, :],
                        lt4[:])

            mlp_t = {}
            NSB = BSEG // P          # 32 segment tiles
            TPS = NSB // NSPLITP     # 8 tiles per pool RS slice
            mdone = [0]
            npcoll = [0]

            def emit_mlp_ready():
                ready = max(0, npcoll[0] - 2) * TPS
                while mdone[0] < ready:
                    emit_mlp(mdone[0])
                    mdone[0] += 1

            pending = []
            gather_fence(zext, SHARD)
            gtH, ecolH = pool_stream(TPh, pool_idx_h, "H")
            gtT, ecolT = pool_stream(TPt, pool_idx_t, "T")
            supH, supT = by_super(mmPh), by_super(mmPt)
            ev2 = None
            for sup in range(BSUP):
                gi = sup % 2
                if gi == 0:
                    ev2 = sb.tile([P, 4, 260], bf16, tag="evp")
                for (pn, sups, gts, ecol, loc_t, c0) in (
                        ("H", supH, gtH, ecolH, pool_loc_h, 0),
                        ("T", supT, gtT, ecolT, pool_loc_t, 130)):
                    pp = psp.tile([P, 2, 130], f32, tag=f"pp{pn}{sup % 2}",
                                  name=f"psp{pn}{sup % 2}")
                    touched = [False, False]
                    for (j, ch, b, st, sp) in sups.get(sup, []):
                        ohw_t = ohp.tile([P, P], bf16, tag="ohwp", bufs=8,
                                         name="ohw")
                        nc.vector.tensor_scalar(
                            out=ohw_t[:], in0=iota_b[:],
                            scalar1=loc_t[:, j:j + 1],
                            scalar2=ecol[:, ch:ch + 1],
                            op0=mybir.AluOpType.is_equal,
                            op1=mybir.AluOpType.mult)
                        h = b % 2
                        touched[h] = True
                        nc.tensor.matmul(
                            pp[:, h, :], lhsT=ohw_t[:],
                            rhs=gts[ch // (GBP // P)][:, ch % (GBP // P), 0:130],
                            start=st, stop=sp)
                    assert touched[0] and touched[1], "empty pool bucket"
                    nc.scalar.activation(
                        ev2[:, gi * 2:gi * 2 + 2, c0:c0 + 130],
                        pp[:],
                        mybir.ActivationFunctionType.Copy)
                if gi == 1:
                    g = sup // 2
                    nc.sync.dma_start(
                        ppart[:].rearrange("(g q p2) d -> p2 g q d",
                                           q=4, p2=P)[:, g, :, :],
                        ev2[:])
                if sup + 1 in PSUPB:
                    sl = PSUPB[sup + 1]
                    r0 = PSUPCUM[sl] * 2 * P
                    r1 = PSUPCUM[sl + 1] * 2 * P
                    n0 = PCUM[sl] * 256
                    n1 = PCUM[sl + 1] * 256
                    pending.append((ppart[:][r0:r1, :], pout[:][n0:n1, :]))
                if pending and sup in PSUPTRIG:
                    coll(*pending.pop(0))
            for args in pending:
                coll(*args)

        # ---------------- normalize + MLP (my 4096 segments) ----------------
        with tc.tile_pool(name="psm", bufs=2, space="PSUM") as psm:
            psm_holder["psm"] = psm
            for t in range(NSB):
                emit_mlp(t)

    nc.compile()
    return nc


def kernel(embed, temp, attn_w, attn_b, W1, b1, W2, b2,
           edge_index, H_idx, H_seg, T_idx, T_seg, B):
    embed = np.asarray(embed, np.float32)
    temp = np.asarray(temp, np.float32)
    attn_w = np.asarray(attn_w, np.float32)
    W1 = np.asarray(W1, np.float32)
    b1 = np.asarray(b1, np.float32)
    W2 = np.asarray(W2, np.float32)
    b2 = np.asarray(b2, np.float32)
    edge_index = np.asarray(edge_index)
    H_idx, H_seg = np.asarray(H_idx), np.asarray(H_seg)
    T_idx, T_seg = np.asarray(T_idx), np.asarray(T_seg)

    S = _make_schedules(edge_index, H_idx, H_seg, T_idx, T_seg)
    TH, mmH = S["TH"], S["mmH"]
    TPh, mmPh, TPt, mmPt = S["TPh"], S["mmPh"], S["TPt"], S["mmPt"]

    key = (TH, len(mmH), TPh, len(mmPh), TPt, len(mmPt))
    if key not in _COMPILED:
        _COMPILED[key] = _build_program(TH, mmH, len(mmPh), mmPh,
                                        len(mmPt), mmPt, TPh, TPt)
    nc = _COMPILED[key]

    def pair_layout_f32(v, c):
        # [p, t, h] for nodes (t*128+p)*2+h of core c
        lo = c * SHARD
        arr = v[lo:lo + SHARD].reshape(NTILE, P, 2)
        return np.ascontiguousarray(arr.transpose(1, 0, 2).astype(np.float32))

    bf = ml_dtypes.bfloat16
    in_maps = []
    for c in range(NCORES):
        lo = c * SHARD
        n_real = max(0, min(SHARD, N - lo))
        esh = np.zeros((SHARD, D), np.float32)
        esh[:n_real] = embed[lo:lo + n_real]
        wr = np.tile(attn_w[:, 0][None, :], (P, 2))
        in_maps.append(dict(
            embed_bf=esh.astype(bf),
            bsc_in=pair_layout_f32(S["b_sc"], c),
            temp_in=np.tile(temp[None, :], (P, 1)),
            wrep_in=wr,
            w1_in=W1.reshape(3, P, H_MLP).astype(bf),
            b1_in=np.ascontiguousarray(b1.reshape(4, P).T),
            w2_in=W2.reshape(4, P, R).astype(bf),
            b2_in=b2[:, None].copy(),
            hsrc=_wrap_idx16(S["hidxs"][c]),
            hloc=S["hlocs"][c],
            hw0=S["hw0s"][c],
            hw12=S["hw12s"][c],
            psrcH=_wrap_idx16(S["pidxsH"][c]),
            plocH=S["plocsH"][c],
            psrcT=_wrap_idx16(S["pidxsT"][c]),
            plocT=S["plocsT"][c],
        ))

    res = run_bass_kernel_spmd(nc, in_maps, list(range(NCORES)))
    return np.concatenate([res.results[c]["out"] for c in range(NCORES)], axis=0)
